# revision 5
# baseline (speedup 1.0000x reference)
"""BiMamba2DFast kernel for 8 Trainium2 NeuronCores (Bass/Tile).

Strategy: data-parallel over the 112 (b, w)-sequences — 14 per core, end-to-end
through both mamba stages with no cross-core communication. Each stage runs the
forward and flipped streams as two 784-token chunks. The selective scan runs as
hardware tensor_tensor_scan instructions (128 channel-lanes x 784 tokens),
segmented at sequence starts by poisoning dt so exp(A*dt) = 0 there. The
inter-stage (d,h)->(i,j) memory reinterpretation bounces through a DRAM scratch
tile; flipped stage-2 streams are built with PE transposes against an
anti-identity so no negative-stride DMA is needed.
"""
import sys

sys.path.insert(0, '/opt/trn_rl_repo')

import numpy as np
import concourse.bass as bass
import concourse.tile as tile
from concourse import mybir
from concourse.bass_utils import run_bass_kernel_spmd
import bass_rust

f32 = mybir.dt.float32
Alu = mybir.AluOpType
Act = mybir.ActivationFunctionType

DM, DI, DS, DR = 192, 384, 16, 12          # d_model, d_inner, d_state, dt_rank
B, H, W, L = 2, 56, 56, 56
NSEQ = B * W                               # 112 sequences total
NCORE = 8
SPC = NSEQ // NCORE                        # 14 sequences per core
NTOK = SPC * L                             # 784 tokens per chunk (fwd or rev)
TC = 392                                   # matmul token chunk (7 seqs, <=512)
NTC = NTOK // TC                           # 2
PADS = 58                                  # conv buffer stride per seq (2 pads + 56)
POISON = 1e30

_nop_ctr = [0]


def _make_wait_nop(engine, wait):
    _nop_ctr[0] += 1
    inst = bass_rust.InstNoOp(name=f"waitnop-{_nop_ctr[0]}", hint="splitwait",
                              cycle_cnt=0)
    inst.engine = engine
    inst.sync_info = bass_rust.SyncInfo(on_wait=[wait], on_update=[])
    return inst


def split_excess_waits(nc, max_waits=1):
    """This walrus build rejects >max_waits sem waits per instruction; hoist
    the excess onto same-engine NoOps placed just before the instruction."""
    for fn in nc.m.functions:
        for bb in fn.blocks:
            if not any(inst.sync_info is not None and inst.sync_info.on_wait
                       and len(inst.sync_info.on_wait) > max_waits
                       for inst in bb.instructions):
                continue
            new_list = []
            for inst in bb.instructions:
                si = inst.sync_info
                if si is not None and si.on_wait and len(si.on_wait) > max_waits:
                    waits = list(si.on_wait)
                    keep = waits[-max_waits:]
                    for w in waits[:-max_waits]:
                        new_list.append(_make_wait_nop(inst.engine, w))
                    si.on_wait = keep
                new_list.append(inst)
            bb.instructions[:] = new_list


def build_nc():
    nc = bass.Bass()

    def din(nm, sh):
        return nc.declare_dram_parameter(nm, list(sh), f32, isOutput=False)

    x_t = din("x_t", (DM, 2 * NTOK))
    wt_dram = {}
    for p in ("h", "w"):
        wt_dram[p] = dict(
            inw=din(f"{p}_inw", (DM, 2 * DI)),
            xpw=din(f"{p}_xpw", (DI, DR + 2 * DS)),
            dtw=din(f"{p}_dtw", (DR, DI)),
            outw=din(f"{p}_outw", (DI, DM)),
            convw=din(f"{p}_convw", (DI, 3)),
            convb=din(f"{p}_convb", (DI, 1)),
            dtb=din(f"{p}_dtb", (DI, 1)),
            A=din(f"{p}_A", (DI, DS)),
            D=din(f"{p}_D", (DI, 1)),
        )
    ident_d = din("ident", (128, 128))
    selB_d = din("selB", (DR + 2 * DS, DS * 128))
    selC_d = din("selC", (DR + 2 * DS, DS * 128))
    jrev_d = din("jrev", (L, L))
    out_d = nc.declare_dram_parameter("out", [DM, NTOK], f32, isOutput=True)

    with tile.TileContext(nc) as tc:
        with (
            tc.tile_pool(name="pers", bufs=1) as pers,
            tc.tile_pool(name="work", bufs=1) as work,
            tc.tile_pool(name="trans", bufs=2) as trans,
            tc.tile_pool(name="trans1", bufs=1) as trans1,
            tc.tile_pool(name="psum", bufs=7, space=bass.MemorySpace.PSUM) as psum,
            tc.tile_pool(name="dram", bufs=1, space="DRAM") as dpool,
        ):
            o1 = dpool.tile([SPC, DM, L], f32, tag="o1")

            # ---- persistent small tensors ----
            identsb = pers.tile([128, 128], f32, tag="ident")
            nc.sync.dma_start(identsb[:], ident_d[:])
            jrevsb = pers.tile([L, L], f32, tag="jrev")
            nc.sync.dma_start(jrevsb[:], jrev_d[:])
            selB = pers.tile([DR + 2 * DS, DS * 128], f32, tag="selB")
            nc.sync.dma_start(selB[:], selB_d[:])
            selC = pers.tile([DR + 2 * DS, DS * 128], f32, tag="selC")
            nc.sync.dma_start(selC[:], selC_d[:])

            # conv padded buffers (pads zeroed once; data cols rewritten per use)
            xi_pad = []
            for ct in range(3):
                t = pers.tile([128, SPC * PADS], f32, tag=f"xipad{ct}")
                r = t[:].rearrange("p (s q) -> p s q", q=PADS)
                nc.vector.memset(r[:, :, 0:2], 0.0)
                xi_pad.append(t)

            # ---- weights to SBUF ----
            wts = {}
            for p in ("h", "w"):
                d = wt_dram[p]
                w = {}
                w["inw1"] = pers.tile([128, 2 * DI], f32, name=f"{p}inw1", tag=f"{p}inw1")
                nc.sync.dma_start(w["inw1"][:], d["inw"][0:128, :])
                w["inw2"] = pers.tile([64, 2 * DI], f32, name=f"{p}inw2", tag=f"{p}inw2")
                nc.sync.dma_start(w["inw2"][:], d["inw"][128:192, :])
                w["dtw"] = pers.tile([DR, DI], f32, name=f"{p}dtw", tag=f"{p}dtw")
                nc.sync.dma_start(w["dtw"][:], d["dtw"][:])
                for ct in range(3):
                    for nm, sh in (("xpw", (128, DR + 2 * DS)),
                                   ("outw", (128, DM)), ("convw", (128, 3)),
                                   ("convb", (128, 1)), ("dtb", (128, 1)),
                                   ("A", (128, DS)), ("D", (128, 1))):
                        tl = pers.tile(list(sh), f32, name=f"{p}{nm}{ct}", tag=f"{p}{nm}{ct}")
                        nc.sync.dma_start(tl[:], d[nm][ct * 128:(ct + 1) * 128, :])
                        w[f"{nm}{ct}"] = tl
                wts[p] = w

            o1_flat = o1[:].rearrange("s d h -> s (d h)").rearrange(
                "s (i j) -> (s i) j", j=DM)

            def load_stage_input(si, chunk):
                """Return (in1 [128, NTOK], in2 [64, NTOK]) token-major input."""
                in1 = work.tile([128, NTOK], f32, tag="in1")
                in2 = work.tile([64, NTOK], f32, tag="in2")
                if si == 0:
                    c0 = chunk * NTOK
                    nc.sync.dma_start(in1[:], x_t[0:128, c0:c0 + NTOK])
                    nc.sync.dma_start(in2[:], x_t[128:192, c0:c0 + NTOK])
                    return in1, in2
                if chunk == 0:
                    for t in range(7):  # 7 tiles x 112 rows (2 seqs each)
                        x2f = trans.tile([112, DM], f32, tag="x2f")
                        nc.sync.dma_start(x2f[:], o1_flat[t * 112:(t + 1) * 112, :])
                        pt1 = psum.tile([128, 112], f32, tag="ps")
                        nc.tensor.matmul(pt1[:], x2f[:, 0:128],
                                         identsb[0:112, 0:112],
                                         is_transpose=True, start=True, stop=True)
                        nc.scalar.copy(in1[:, t * 112:(t + 1) * 112], pt1[:])
                        pt2 = psum.tile([64, 112], f32, tag="ps")
                        nc.tensor.matmul(pt2[:], x2f[:, 128:192],
                                         identsb[0:112, 0:112],
                                         is_transpose=True, start=True, stop=True)
                        nc.scalar.copy(in2[:, t * 112:(t + 1) * 112], pt2[:])
                else:
                    for s in range(SPC):  # one seq per tile, reversed via jrev
                        x2r = trans.tile([L, DM], f32, tag="x2r")
                        nc.sync.dma_start(x2r[:], o1_flat[s * L:(s + 1) * L, :])
                        pt1 = psum.tile([128, L], f32, tag="ps")
                        nc.tensor.matmul(pt1[:], x2r[:, 0:128], jrevsb[:],
                                         is_transpose=True, start=True, stop=True)
                        nc.scalar.copy(in1[:, s * L:(s + 1) * L], pt1[:])
                        pt2 = psum.tile([64, L], f32, tag="ps")
                        nc.tensor.matmul(pt2[:], x2r[:, 128:192], jrevsb[:],
                                         is_transpose=True, start=True, stop=True)
                        nc.scalar.copy(in2[:, s * L:(s + 1) * L], pt2[:])
                return in1, in2

            def emit_stage(si, w, dst):
                gated = [[None] * 3, [None] * 3]
                for chunk in range(2):
                    in1, in2 = load_stage_input(si, chunk)

                    # ---- in_proj: xz[e, tok] = W_in^T-chunks @ IN ----
                    siluz, xc, dtsp, dtxc, yacc = [], [], [], [], []
                    for e in range(6):
                        for j in range(NTC):
                            t0 = j * TC
                            pm = psum.tile([128, TC], f32, tag="ps")
                            nc.tensor.matmul(
                                pm[:], w["inw1"][:, e * 128:(e + 1) * 128],
                                in1[:, t0:t0 + TC], start=True, stop=False)
                            nc.tensor.matmul(
                                pm[:], w["inw2"][:, e * 128:(e + 1) * 128],
                                in2[:, t0:t0 + TC], start=False, stop=True)
                            if e < 3:  # xi rows -> padded conv buffer
                                r = xi_pad[e][:].rearrange(
                                    "p (s q) -> p s q", q=PADS)
                                nc.scalar.copy(
                                    r[:, 7 * j:7 * j + 7, 2:PADS],
                                    pm[:].rearrange("p (s q) -> p s q", q=L))
                            else:      # z rows -> silu(z)
                                if j == 0:
                                    t = work.tile([128, NTOK], f32,
                                                  tag=f"siluz{e - 3}")
                                    siluz.append(t)
                                nc.scalar.activation(
                                    siluz[e - 3][:, t0:t0 + TC], pm[:], Act.Silu)

                    # ---- causal depthwise conv + silu ----
                    for ct in range(3):
                        r = xi_pad[ct][:].rearrange("p (s q) -> p s q", q=PADS)
                        v0 = r[:, :, 0:L]
                        v1 = r[:, :, 1:1 + L]
                        v2 = r[:, :, 2:2 + L]
                        a = trans1.tile([128, NTOK], f32, tag=f"cvt{ct}")
                        a3 = a[:].rearrange("p (s l) -> p s l", l=L)
                        nc.vector.tensor_scalar(a3, v0, w[f"convw{ct}"][:, 0:1],
                                                None, Alu.mult)
                        nc.vector.scalar_tensor_tensor(
                            a3, v1, w[f"convw{ct}"][:, 1:2], a3,
                            Alu.mult, Alu.add)
                        nc.vector.scalar_tensor_tensor(
                            a3, v2, w[f"convw{ct}"][:, 2:3], a3,
                            Alu.mult, Alu.add)
                        t = work.tile([128, NTOK], f32, tag=f"xc{ct}")
                        nc.scalar.activation(t[:], a[:], Act.Silu,
                                             bias=w[f"convb{ct}"][:, 0:1])
                        xc.append(t)

                    # ---- x_proj: dbl[44, tok] ----
                    dbl = work.tile([DR + 2 * DS, NTOK], f32, tag="dbl")
                    for j in range(NTC):
                        t0 = j * TC
                        pm = psum.tile([DR + 2 * DS, TC], f32, tag="ps")
                        for ct in range(3):
                            nc.tensor.matmul(pm[:], w[f"xpw{ct}"][:],
                                             xc[ct][:, t0:t0 + TC],
                                             start=(ct == 0), stop=(ct == 2))
                        nc.scalar.copy(dbl[:, t0:t0 + TC], pm[:])

                    # ---- dt = softplus(dt_w @ dt_raw + dt_b); dtxc; poison ----
                    for ct in range(3):
                        t = work.tile([128, NTOK], f32, tag=f"dtsp{ct}")
                        for j in range(NTC):
                            t0 = j * TC
                            pm = psum.tile([128, TC], f32, tag="ps")
                            nc.tensor.matmul(pm[:],
                                             w["dtw"][:, ct * 128:(ct + 1) * 128],
                                             dbl[0:DR, t0:t0 + TC],
                                             start=True, stop=True)
                            nc.scalar.activation(t[:, t0:t0 + TC], pm[:], Act.Exp,
                                                 bias=w[f"dtb{ct}"][:, 0:1])
                        nc.scalar.activation(t[:], t[:], Act.Ln, bias=1.0)
                        dtsp.append(t)
                        tx = work.tile([128, NTOK], f32, tag=f"dtxc{ct}")
                        nc.vector.tensor_tensor(tx[:], t[:], xc[ct][:], Alu.mult)
                        dtxc.append(tx)
                        r = t[:].rearrange("p (s l) -> p s l", l=L)
                        nc.vector.memset(r[:, :, 0:1], POISON)
                        ya = work.tile([128, NTOK], f32, tag=f"yacc{ct}")
                        nc.gpsimd.memset(ya[:], 0.0)
                        yacc.append(ya)

                    # ---- selective scan over state index n ----
                    for n in range(DS):
                        pbs = []
                        for j in range(NTC):
                            t0 = j * TC
                            pb = psum.tile([128, TC], f32, tag="ps")
                            nc.tensor.matmul(pb[:],
                                             selB[:, n * 128:(n + 1) * 128],
                                             dbl[:, t0:t0 + TC],
                                             start=True, stop=True)
                            pbs.append(pb)
                        pcs = []
                        for j in range(NTC):
                            t0 = j * TC
                            pc = psum.tile([128, TC], f32, tag="ps")
                            nc.tensor.matmul(pc[:],
                                             selC[:, n * 128:(n + 1) * 128],
                                             dbl[:, t0:t0 + TC],
                                             start=True, stop=True)
                            pcs.append(pc)
                        for ct in range(3):
                            dA = trans.tile([128, NTOK], f32, tag=f"dA{ct}")
                            nc.scalar.activation(dA[:], dtsp[ct][:], Act.Exp,
                                                 scale=w[f"A{ct}"][:, n:n + 1])
                            dBx = trans1.tile([128, NTOK], f32, tag=f"dBx{ct}")
                            for j in range(NTC):
                                t0 = j * TC
                                nc.vector.tensor_tensor(
                                    dBx[:, t0:t0 + TC], dtxc[ct][:, t0:t0 + TC],
                                    pbs[j][:], Alu.mult)
                            hs = trans1.tile([128, NTOK], f32, tag=f"hs{ct}")
                            nc.vector.tensor_tensor_scan(
                                hs[:], dA[:], dBx[:], 0.0, Alu.mult, Alu.add)
                            yt = trans.tile([128, NTOK], f32, tag=f"yt{ct}")
                            for j in range(NTC):
                                t0 = j * TC
                                nc.vector.tensor_tensor(
                                    yt[:, t0:t0 + TC], hs[:, t0:t0 + TC],
                                    pcs[j][:], Alu.mult)
                            nc.gpsimd.tensor_tensor(yacc[ct][:], yacc[ct][:],
                                                    yt[:], Alu.add)

                    # ---- y = (yacc + D*xc) * silu(z) ----
                    for ct in range(3):
                        nc.vector.scalar_tensor_tensor(
                            yacc[ct][:], xc[ct][:], w[f"D{ct}"][:, 0:1],
                            yacc[ct][:], Alu.mult, Alu.add)
                        g = work.tile([128, NTOK], f32, tag=f"g{chunk}{ct}")
                        nc.vector.tensor_tensor(g[:], yacc[ct][:], siluz[ct][:],
                                                Alu.mult)
                        gated[chunk][ct] = g

                # ---- bidirectional sum + out_proj ----
                for ct in range(3):
                    nc.vector.tensor_tensor(gated[0][ct][:], gated[0][ct][:],
                                            gated[1][ct][:], Alu.add)
                for dchunk, dlo, dhi in ((0, 0, 128), (1, 128, 192)):
                    osb = work.tile([dhi - dlo, NTOK], f32, tag=f"osb{dchunk}")
                    for j in range(NTC):
                        t0 = j * TC
                        pm = psum.tile([dhi - dlo, TC], f32, tag="ps")
                        for ct in range(3):
                            nc.tensor.matmul(pm[:], w[f"outw{ct}"][:, dlo:dhi],
                                             gated[0][ct][:, t0:t0 + TC],
                                             start=(ct == 0), stop=(ct == 2))
                        nc.scalar.copy(osb[:, t0:t0 + TC], pm[:])
                    if dst == "o1":
                        dap = o1[:, dlo:dhi, :].transpose([1, 0, 2])
                        nc.sync.dma_start(
                            dap, osb[:].rearrange("p (s l) -> p s l", l=L))
                    else:
                        nc.sync.dma_start(out_d[dlo:dhi, :], osb[:])

            emit_stage(0, wts["h"], "o1")
            emit_stage(1, wts["w"], "out")

    split_excess_waits(nc)
    return nc


_NC_CACHE = None


def _get_nc():
    global _NC_CACHE
    if _NC_CACHE is None:
        _NC_CACHE = build_nc()
    return _NC_CACHE


def kernel(**inputs):
    inputs = {k: np.asarray(v, dtype=np.float32) for k, v in inputs.items()}
    x = inputs["x"]

    selB = np.zeros((DR + 2 * DS, DS * 128), np.float32)
    selC = np.zeros((DR + 2 * DS, DS * 128), np.float32)
    for n in range(DS):
        selB[DR + n, n * 128:(n + 1) * 128] = 1.0
        selC[DR + DS + n, n * 128:(n + 1) * 128] = 1.0
    base = {
        "ident": np.eye(128, dtype=np.float32),
        "jrev": np.fliplr(np.eye(L, dtype=np.float32)).copy(),
        "selB": selB,
        "selC": selC,
    }
    for p, tag in (("h", "h_"), ("w", "w_")):
        base[f"{p}_inw"] = np.ascontiguousarray(inputs[tag + "in_proj_w"].T)
        base[f"{p}_xpw"] = np.ascontiguousarray(inputs[tag + "x_proj_w"].T)
        base[f"{p}_dtw"] = np.ascontiguousarray(inputs[tag + "dt_proj_w"].T)
        base[f"{p}_outw"] = np.ascontiguousarray(inputs[tag + "out_proj_w"].T)
        base[f"{p}_convw"] = np.ascontiguousarray(inputs[tag + "conv_w"])
        base[f"{p}_convb"] = inputs[tag + "conv_b"].reshape(DI, 1).copy()
        base[f"{p}_dtb"] = inputs[tag + "dt_proj_b"].reshape(DI, 1).copy()
        base[f"{p}_A"] = np.ascontiguousarray(-np.exp(inputs[tag + "A_log"]))
        base[f"{p}_D"] = inputs[tag + "D_skip"].reshape(DI, 1).copy()

    in_maps = []
    for core in range(NCORE):
        sl = range(core * SPC, (core + 1) * SPC)
        seqs = np.stack([x[s // W, :, s % W, :] for s in sl])    # [14, 56, 192]
        fwd = seqs.reshape(NTOK, DM)
        rev = seqs[:, ::-1, :].reshape(NTOK, DM)
        x_t = np.ascontiguousarray(
            np.concatenate([fwd, rev], axis=0).T)                # [192, 1568]
        m = dict(base)
        m["x_t"] = x_t
        in_maps.append(m)

    nc = _get_nc()
    res = run_bass_kernel_spmd(nc, in_maps, core_ids=list(range(NCORE)))

    out_full = np.zeros((NSEQ, L, DM), np.float32)
    for core in range(NCORE):
        o = res.results[core]["out"]                             # [192, 784]
        out_full[core * SPC:(core + 1) * SPC] = o.T.reshape(SPC, L, DM)
    return out_full.reshape(B, H, W, DM)
